# revision 15
# baseline (speedup 1.0000x reference)
"""Allpass biquad IIR filter (torchaudio allpass_biquad semantics) on 8 TRN2 cores.

Input x: [64, 1, 480000] f32.  y[n] = B0 x[n] + B1 x[n-1] + B2 x[n-2] - A1 y[n-1] - A2 y[n-2].

With sr=16000, f0=4000: w0 = pi/2, so cos(w0) ~ 6e-17 and B1 = A1 ~ -7e-17 —
negligible at f32 precision (rel ~1e-16). Also B2 = (1+a)/(1+a) = 1.0 exactly.
The recurrence therefore splits into independent even/odd first-order streams:

    y[n] = B0 x[n] + x[n-2] - A2 y[n-2]

Substituting z[n] = y[n] - B0 x[n]:

    z[n] = c z[n-2] + q x[n-2],   c = -A2, q = 1 - A2*B0 = 1 - A2^2
    y[n] = z[n] + B0 x[n]

which maps directly onto the DVE's tensor_tensor_scan (state = d0*state + d1)
run with stride-2 access patterns for the even/odd phases.

Sharding: pure data parallel — 8 sequences per core. Each sequence is further
split into 16 row-segments of 30000 samples so all 128 SBUF partitions carry
independent work. |c| = 0.17 so the IIR memory is ~16 taps: a 32-sample halo
from the preceding segment (zeros at sequence start) warms up the scan state to
below f32 noise (c^16 ~ 6e-13), making all 128 rows fully independent.
"""

import math

import numpy as np

# ---- fixed problem geometry ----
N_SEQ = 64
T = 480000
N_CORES = 8
SEQ_PER_CORE = N_SEQ // N_CORES  # 8
SEGS_PER_SEQ = 16
P = SEQ_PER_CORE * SEGS_PER_SEQ  # 128 partitions
SEG = T // SEGS_PER_SEQ  # 30000 samples per row
F = 3750  # columns per tile step
H = 32  # decay halo (even); c^(H/2) ~ 6e-13
HP2 = H + 2  # halo incl. 2-col FIR lookback


def _coeffs():
    w0 = 2.0 * math.pi * 4000.0 / 16000.0
    alpha = math.sin(w0) / (2.0 * 0.707)
    a0 = 1.0 + alpha
    b0 = np.float32((1.0 - alpha) / a0)
    a2 = np.float32((1.0 - alpha) / a0)
    c = np.float32(-float(a2))
    q = np.float32(1.0 - float(a2) * float(b0))
    return b0, c, q


def build(P=P, SEG=SEG, F=F, H=H):
    """Build the per-core Bass graph (SPMD: same program on all 8 cores)."""
    import concourse.tile as tile
    from concourse import bacc, mybir

    B0f, Cf, Qf = _coeffs()
    HP2 = H + 2
    NT = SEG // F
    assert SEG % F == 0 and F % 2 == 0 and H % 2 == 0

    nc = bacc.Bacc()
    x = nc.declare_dram_parameter("x", [P, SEG], mybir.dt.float32, isOutput=False)
    # per-row left halo, host-prepared: predecessor row's tail, zeros at
    # sequence starts
    halo = nc.declare_dram_parameter("halo", [P, HP2], mybir.dt.float32, isOutput=False)
    out = nc.declare_dram_parameter("out", [P, SEG], mybir.dt.float32, isOutput=True)

    with tile.TileContext(nc) as tc:
        with (
            tc.tile_pool(name="xp", bufs=3) as xp,
            tc.tile_pool(name="qp", bufs=2) as qp,
            tc.tile_pool(name="bp", bufs=2) as bp,
            tc.tile_pool(name="zp", bufs=2) as zp,
            tc.tile_pool(name="yp", bufs=2) as yp,
            tc.tile_pool(name="cp", bufs=1) as cp,
        ):
            # constant multiplier tile for the scan's data0 (state coefficient)
            ctile = cp.tile([P, (H + F) // 2], mybir.dt.float32, tag="c")
            nc.gpsimd.memset(ctile[:], float(Cf))

            zprev = None
            for k in range(NT):
                u = HP2 + F if k == 0 else 2 + F
                xt = xp.tile([P, HP2 + F], mybir.dt.float32, tag="x")
                if k == 0:
                    nc.sync.dma_start(xt[:, 0:HP2], halo[:])
                    nc.sync.dma_start(xt[:, HP2 : HP2 + F], x[:, 0:F])
                else:
                    nc.sync.dma_start(xt[:, 0:u], x[:, k * F - 2 : (k + 1) * F])

                # qx = q * x and bx = B0 * x   (ScalarE, keeps DVE free)
                qt = qp.tile([P, HP2 + F], mybir.dt.float32, tag="q")
                nc.scalar.activation(
                    qt[:, 0:u],
                    xt[:, 0:u],
                    mybir.ActivationFunctionType.Copy,
                    scale=float(Qf),
                )
                xoff = HP2 if k == 0 else 2
                bt = bp.tile([P, F], mybir.dt.float32, tag="b")
                nc.scalar.activation(
                    bt[:],
                    xt[:, xoff : xoff + F],
                    mybir.ActivationFunctionType.Copy,
                    scale=float(B0f),
                )

                # z scan: even/odd phases, each a first-order recurrence
                zw = H + F if k == 0 else F  # z cols map to times [kF - (k==0)*H, (k+1)F)
                zt = zp.tile([P, H + F], mybir.dt.float32, tag="z")
                for ph in range(2):
                    if k == 0:
                        init = 0.0
                    else:
                        pw = H + F if k == 1 else F
                        init = zprev[:, pw - 2 + ph : pw - 1 + ph]
                    nc.vector.tensor_tensor_scan(
                        out=zt[:, ph:zw:2],
                        data0=ctile[:, 0 : zw // 2],
                        data1=qt[:, ph:zw:2],
                        initial=init,
                        op0=mybir.AluOpType.mult,
                        op1=mybir.AluOpType.add,
                    )

                # y = bx + z   (GpSimd tensor_tensor: keeps DVE free)
                yt = yp.tile([P, F], mybir.dt.float32, tag="y")
                zoff = H if k == 0 else 0
                nc.gpsimd.tensor_tensor(
                    out=yt[:],
                    in0=bt[:],
                    in1=zt[:, zoff : zoff + F],
                    op=mybir.AluOpType.add,
                )
                nc.sync.dma_start(out[:, k * F : (k + 1) * F], yt[:])
                zprev = zt
    nc.finalize()
    return nc


def _shard(x):
    """x: [64, 1, 480000] f32 -> list of 8 per-core input maps."""
    in_maps = []
    for i in range(N_CORES):
        shard = np.ascontiguousarray(
            x[i * SEQ_PER_CORE : (i + 1) * SEQ_PER_CORE, 0, :]
        ).reshape(P, SEG)
        halo = np.zeros((P, HP2), np.float32)
        halo[1:] = shard[:-1, SEG - HP2 :]
        halo[::SEGS_PER_SEQ] = 0.0  # sequence starts: rest state
        in_maps.append({"x": shard, "halo": halo})
    return in_maps


def _unshard(results):
    outs = [
        np.asarray(results[i]["out"]).reshape(SEQ_PER_CORE, T) for i in range(N_CORES)
    ]
    return np.concatenate(outs, axis=0)[:, None, :].astype(np.float32)


def _install_ntff_hook_shim():
    """This image's `antenv` lacks `axon_hooks`; register the NTFF profile
    hook module ourselves so trace=True works under axon."""
    import sys
    import types

    try:
        import antenv.axon_hooks  # noqa: F401

        return
    except ImportError:
        pass
    try:
        import antenv
        from trn_agent_boot.trn_boot import _ntff_profile_via_ctypes
    except ImportError:
        return

    state = {"hook": None}

    def set_axon_ntff_profile_hook(h):
        state["hook"] = h

    def get_axon_ntff_profile_hook():
        if state["hook"] is None:
            try:
                state["hook"] = _ntff_profile_via_ctypes("/opt/axon/libaxon_pjrt.so")
            except Exception:
                return None
        return state["hook"]

    mod = types.ModuleType("antenv.axon_hooks")
    mod.set_axon_ntff_profile_hook = set_axon_ntff_profile_hook
    mod.get_axon_ntff_profile_hook = get_axon_ntff_profile_hook
    sys.modules["antenv.axon_hooks"] = mod
    antenv.axon_hooks = mod


def run(x, trace=False):
    """Returns (y, BassKernelResults)."""
    import concourse.bass_utils as bass_utils

    if trace:
        _install_ntff_hook_shim()

    x = np.asarray(x)
    assert x.shape == (N_SEQ, 1, T), x.shape
    nc = build()
    res = bass_utils.run_bass_kernel_spmd(
        nc, _shard(x), core_ids=list(range(N_CORES)), trace=trace
    )
    return _unshard(res.results), res


def kernel(x):
    y, _ = run(x, trace=False)
    return y


# revision 18
# speedup vs baseline: 1.3593x; 1.3593x over previous
"""Allpass biquad IIR filter (torchaudio allpass_biquad semantics) on 8 TRN2 cores.

Input x: [64, 1, 480000] f32.  y[n] = B0 x[n] + B1 x[n-1] + B2 x[n-2] - A1 y[n-1] - A2 y[n-2].

With sr=16000, f0=4000: w0 = pi/2, so cos(w0) ~ 6e-17 and B1 = A1 ~ -7e-17 —
negligible at f32 precision (rel ~1e-16). Also B2 = (1+a)/(1+a) = 1.0 exactly.
The recurrence therefore splits into independent even/odd first-order streams:

    y[n] = B0 x[n] + x[n-2] - A2 y[n-2]

Substituting z[n] = y[n] - B0 x[n]:

    z[n] = c z[n-2] + q x[n-2],   c = -A2, q = 1 - A2*B0 = 1 - A2^2
    y[n] = z[n] + B0 x[n]

which maps directly onto the DVE's tensor_tensor_scan (state = d0*state + d1)
run with stride-2 access patterns for the even/odd phases.

Sharding: pure data parallel — 8 sequences per core. Each sequence is further
split into 16 row-segments of 30000 samples so all 128 SBUF partitions carry
independent work. |c| = 0.17 so the IIR memory is ~16 taps: a 32-sample halo
from the preceding segment (zeros at sequence start) warms up the scan state to
below f32 noise (c^16 ~ 6e-13), making all 128 rows fully independent.
"""

import math

import numpy as np

# ---- fixed problem geometry ----
N_SEQ = 64
T = 480000
N_CORES = 8
SEQ_PER_CORE = N_SEQ // N_CORES  # 8
SEGS_PER_SEQ = 16
P = SEQ_PER_CORE * SEGS_PER_SEQ  # 128 partitions
SEG = T // SEGS_PER_SEQ  # 30000 samples per row
F = 3750  # columns per tile step
H = 32  # decay halo (even); c^(H/2) ~ 6e-13
HP2 = H + 2  # halo incl. 2-col FIR lookback


def _coeffs():
    w0 = 2.0 * math.pi * 4000.0 / 16000.0
    alpha = math.sin(w0) / (2.0 * 0.707)
    a0 = 1.0 + alpha
    b0 = np.float32((1.0 - alpha) / a0)
    a2 = np.float32((1.0 - alpha) / a0)
    c = np.float32(-float(a2))
    q = np.float32(1.0 - float(a2) * float(b0))
    return b0, c, q


def build(P=P, SEG=SEG, F=F, H=H):
    """Build the per-core Bass graph (SPMD: same program on all 8 cores)."""
    import concourse.tile as tile
    from concourse import bacc, mybir

    B0f, Cf, Qf = _coeffs()
    HP2 = H + 2
    NT = SEG // F
    assert SEG % F == 0 and F % 2 == 0 and H % 2 == 0

    nc = bacc.Bacc()
    x = nc.declare_dram_parameter("x", [P, SEG], mybir.dt.float32, isOutput=False)
    # per-row left halo, host-prepared: predecessor row's tail, zeros at
    # sequence starts
    halo = nc.declare_dram_parameter("halo", [P, HP2], mybir.dt.float32, isOutput=False)
    out = nc.declare_dram_parameter("out", [P, SEG], mybir.dt.float32, isOutput=True)

    with tile.TileContext(nc) as tc:
        with (
            tc.tile_pool(name="xp", bufs=3) as xp,
            tc.tile_pool(name="qp", bufs=3) as qp,
            tc.tile_pool(name="zp", bufs=2) as zp,
            tc.tile_pool(name="yp", bufs=2) as yp,
            tc.tile_pool(name="cp", bufs=1) as cp,
        ):
            # constant multiplier tile for the scan's data0 (state coefficient)
            ctile = cp.tile([P, (H + F) // 2], mybir.dt.float32, tag="c")
            nc.gpsimd.memset(ctile[:], float(Cf))

            zprev = None
            for k in range(NT):
                u = HP2 + F if k == 0 else 2 + F
                xt = xp.tile([P, HP2 + F], mybir.dt.float32, tag="x")
                if k == 0:
                    nc.sync.dma_start(xt[:, 0:HP2], halo[:])
                    nc.sync.dma_start(xt[:, HP2 : HP2 + F], x[:, 0:F])
                else:
                    nc.sync.dma_start(xt[:, 0:u], x[:, k * F - 2 : (k + 1) * F])

                # qx = q * x   (ScalarE, keeps DVE free)
                qt = qp.tile([P, HP2 + F], mybir.dt.float32, tag="q")
                nc.scalar.activation(
                    qt[:, 0:u],
                    xt[:, 0:u],
                    mybir.ActivationFunctionType.Copy,
                    scale=float(Qf),
                )

                # z scan: even/odd phases, each a first-order recurrence
                zw = H + F if k == 0 else F  # z cols map to times [kF - (k==0)*H, (k+1)F)
                zt = zp.tile([P, H + F], mybir.dt.float32, tag="z")
                for ph in range(2):
                    if k == 0:
                        init = 0.0
                    else:
                        pw = H + F if k == 1 else F
                        init = zprev[:, pw - 2 + ph : pw - 1 + ph]
                    nc.vector.tensor_tensor_scan(
                        out=zt[:, ph:zw:2],
                        data0=ctile[:, 0 : zw // 2],
                        data1=qt[:, ph:zw:2],
                        initial=init,
                        op0=mybir.AluOpType.mult,
                        op1=mybir.AluOpType.add,
                    )

                # y = B0 * x + z
                yt = yp.tile([P, F], mybir.dt.float32, tag="y")
                xoff = HP2 if k == 0 else 2
                zoff = H if k == 0 else 0
                nc.vector.scalar_tensor_tensor(
                    out=yt[:],
                    in0=xt[:, xoff : xoff + F],
                    scalar=float(B0f),
                    in1=zt[:, zoff : zoff + F],
                    op0=mybir.AluOpType.mult,
                    op1=mybir.AluOpType.add,
                )
                nc.sync.dma_start(out[:, k * F : (k + 1) * F], yt[:])
                zprev = zt
    nc.finalize()
    return nc


def _shard(x):
    """x: [64, 1, 480000] f32 -> list of 8 per-core input maps."""
    in_maps = []
    for i in range(N_CORES):
        shard = np.ascontiguousarray(
            x[i * SEQ_PER_CORE : (i + 1) * SEQ_PER_CORE, 0, :]
        ).reshape(P, SEG)
        halo = np.zeros((P, HP2), np.float32)
        halo[1:] = shard[:-1, SEG - HP2 :]
        halo[::SEGS_PER_SEQ] = 0.0  # sequence starts: rest state
        in_maps.append({"x": shard, "halo": halo})
    return in_maps


def _unshard(results):
    outs = [
        np.asarray(results[i]["out"]).reshape(SEQ_PER_CORE, T) for i in range(N_CORES)
    ]
    return np.concatenate(outs, axis=0)[:, None, :].astype(np.float32)


def _install_ntff_hook_shim():
    """This image's `antenv` lacks `axon_hooks`; register the NTFF profile
    hook module ourselves so trace=True works under axon."""
    import sys
    import types

    try:
        import antenv.axon_hooks  # noqa: F401

        return
    except ImportError:
        pass
    try:
        import antenv
        from trn_agent_boot.trn_boot import _ntff_profile_via_ctypes
    except ImportError:
        return

    state = {"hook": None}

    def set_axon_ntff_profile_hook(h):
        state["hook"] = h

    def get_axon_ntff_profile_hook():
        if state["hook"] is None:
            try:
                state["hook"] = _ntff_profile_via_ctypes("/opt/axon/libaxon_pjrt.so")
            except Exception:
                return None
        return state["hook"]

    mod = types.ModuleType("antenv.axon_hooks")
    mod.set_axon_ntff_profile_hook = set_axon_ntff_profile_hook
    mod.get_axon_ntff_profile_hook = get_axon_ntff_profile_hook
    sys.modules["antenv.axon_hooks"] = mod
    antenv.axon_hooks = mod


def run(x, trace=False):
    """Returns (y, BassKernelResults)."""
    import concourse.bass_utils as bass_utils

    if trace:
        _install_ntff_hook_shim()

    x = np.asarray(x)
    assert x.shape == (N_SEQ, 1, T), x.shape
    nc = build()
    res = bass_utils.run_bass_kernel_spmd(
        nc, _shard(x), core_ids=list(range(N_CORES)), trace=trace
    )
    return _unshard(res.results), res


def kernel(x):
    y, _ = run(x, trace=False)
    return y
